# revision 11
# baseline (speedup 1.0000x reference)
"""GCN layer (normalized adjacency @ features -> linear -> relu) on 8 TRN2 NeuronCores.

Strategy (row-sharded, 1D node partition; host does layout/dtype prep only):
  - Host shards adj by rows (P=1024 rows/core), adds the identity diagonal,
    TRANSPOSES the shard to adjT [N, P] and casts to fp8_e4m3 (adj values are
    {0,1,2} -- exact in fp8). Layout [j_within_stripe=128, stripe=64, i=1024]
    so each DMA chunk reads 8KB-contiguous lines per partition. 8MB/core.
  - Features are host-cast to bf16 in [p=128, t=64, f=128] layout (2MB).
  - Device: adjT streams in via HWDGE; the PE computes row sums with a
    DoubleRow fp8 ones-matmul pass (contracts the partition axis, 2 stripes
    per MM) chasing the DMA. Two half-column AllGathers exchange row sums;
    d = rsqrt(r) via Sqrt+reciprocal+Newton. d-scaled bf16 features (lhsT)
    x fp8 adjT (rhs) mixed-dtype matmuls accumulate out_pre.T in PSUM.
  - Epilogue per 512-col chunk: PSUM->SBUF copy, fp32 matmul with W.T
    (host-pretransposed), per-row d scale + bias + relu, per-stripe output DMA.
    Host concatenates the 8 [P,128] outputs.
"""

import numpy as np
import ml_dtypes

import concourse.bass as bass
import concourse.bacc as bacc
import concourse.mybir as mybir
import concourse.tile as tile
from concourse.bass_utils import run_bass_kernel_spmd

F32 = mybir.dt.float32
BF16 = mybir.dt.bfloat16
FP8 = mybir.dt.float8e4

N_FULL = 8192
F_DIM = 128
NUM_CORES = 8


def build_kernel(P=1024, N=8192, F=128, num_cores=8):
    assert P == 1024 and N == 8192 and F == 128
    n_st = P // 128          # 8 output stripes per core
    n_jb = N // 128          # 64 j-stripes (contraction)
    NCH = 4                  # adjT DMA chunks
    jpc = n_jb // NCH        # 8 j-stripes per chunk
    CH2 = 512                # output column chunk (PSUM bank)
    n_g = 2                  # gather groups = local-column halves
    spg = n_st // n_g        # 4 output stripes per group

    nc = bacc.Bacc("TRN2", target_bir_lowering=False, debug=False,
                   num_devices=num_cores)

    adjT_h = nc.declare_dram_parameter("adjT8", [128, n_jb, P], FP8,
                                       isOutput=False)
    feat_h = nc.declare_dram_parameter("feat16", [128, n_jb, F], BF16,
                                       isOutput=False)
    wt_h = nc.declare_dram_parameter("wt", [F, F], F32, isOutput=False)
    bias_h = nc.declare_dram_parameter("bias_b", [128, F], F32, isOutput=False)
    ones_h = nc.declare_dram_parameter("ones8", [128, 2, 16], FP8,
                                       isOutput=False)
    eye_h = nc.declare_dram_parameter("eye32", [128, 128], F32, isOutput=False)
    out_h = nc.declare_dram_parameter("out", [P, F], F32, isOutput=True)

    r_loc = nc.dram_tensor("r_local", [1, P], F32)
    r_ful = nc.dram_tensor("r_full", [num_cores, P], F32,
                           addr_space="Shared")

    out_ap = out_h.ap().rearrange("(s p) f -> p s f", p=128)

    def gather():
        nc.gpsimd.collective_compute(
            "AllGather", mybir.AluOpType.bypass,
            replica_groups=[list(range(num_cores))],
            ins=[r_loc[:].opt()],
            outs=[r_ful[:].opt()],
        )

    with tile.TileContext(nc) as tc:
        with tc.tile_pool(name="const", bufs=1) as cpool, \
             tc.tile_pool(name="big", bufs=1) as bigp, \
             tc.tile_pool(name="psB", bufs=1, space="PSUM") as psB:

            # ones8 FIRST on the SP ring: same-ring FIFO guarantees it lands
            # before the adjT flood (a parallel ring's small DMAs were
            # observed to starve until the flood drained). Other consts are
            # not needed until after the flood -> scalar ring.
            ones8 = cpool.tile([128, 2, 16], FP8)
            nc.sync.dma_start(ones8, ones_h[:])
            wt_sb = cpool.tile([F, F], F32)
            nc.scalar.dma_start(wt_sb, wt_h[:])
            bias_bc = cpool.tile([128, F], F32)
            nc.scalar.dma_start(bias_bc, bias_h[:])
            eye32 = cpool.tile([128, 128], F32)
            nc.scalar.dma_start(eye32, eye_h[:])
            # pre-warm the Sqrt activation table (first use pays ~2.7us)
            warm = cpool.tile([1, 1], F32)
            nc.scalar.activation(warm, eye32[0:1, 0:1],
                                 mybir.ActivationFunctionType.Sqrt)

            adjT = bigp.tile([128, n_jb, P], FP8)
            feat16 = bigp.tile([128, n_jb, F], BF16)
            out_sb = bigp.tile([128, n_st, F], F32)

            # main-matmul accumulators (persist across phases)
            pm = [psB.tile([128, CH2], F32, tag=f"pm{h}", name=f"pm{h}")
                  for h in range(2)]

            with tc.tile_pool(name="ph1", bufs=1) as p1, \
                 tc.tile_pool(name="psA", bufs=1, space="PSUM") as psA:
                # adjT chunk DMAs (SP ring) interleaved in program order with
                # the DoubleRow fp8 ones-pass (r[i] = sum_j adjT[j, i], two
                # stripes per MM) so the pass chases the DMA chunk by chunk.
                pr = [psA.tile([1, CH2], F32, tag=f"pr{g}", name=f"pr{g}")
                      for g in range(n_g)]
                # adjT chunk DMAs interleaved with the ones-pass so the
                # pass chases the flood; both column-half accumulations
                # interleave per pair.
                for k in range(NCH):
                    nc.sync.dma_start(adjT[:, k * jpc:(k + 1) * jpc, :],
                                      adjT_h[:, k * jpc:(k + 1) * jpc, :])
                    for pair in range(k * jpc // 2, (k + 1) * jpc // 2):
                        for g in range(n_g):
                            nc.tensor.matmul(
                                pr[g], lhsT=ones8[:, :, 0:1],
                                rhs=adjT[:, 2 * pair:2 * pair + 2,
                                         CH2 * g:CH2 * (g + 1)],
                                start=(pair == 0),
                                stop=(pair == n_jb // 2 - 1),
                                perf_mode=mybir.MatmulPerfMode.DoubleRow)
                # features stream after adjT on the same FIFO ring (needed
                # only once d arrives, well after the gathers fire)
                nc.sync.dma_start(feat16, feat_h[:])
                r_sb = p1.tile([1, P], F32, tag="rsb", name="rsb")
                for g in range(n_g):
                    nc.scalar.copy(r_sb[:, CH2 * g:CH2 * (g + 1)], pr[g])
                nc.scalar.dma_start(r_loc[:], r_sb)
                gather()

            with tc.tile_pool(name="ph2", bufs=1) as p2, \
                 tc.tile_pool(name="psC", bufs=1, space="PSUM") as psC:

                def rsqrt_newton(r_in, width, nm):
                    sq = p2.tile([128, width], F32, tag=f"sq{nm}", name=f"sq{nm}")
                    nc.scalar.activation(sq, r_in,
                                         mybir.ActivationFunctionType.Sqrt)
                    y0 = p2.tile([128, width], F32, tag=f"y0{nm}", name=f"y0{nm}")
                    nc.vector.reciprocal(y0, sq)
                    yy = p2.tile([128, width], F32, tag=f"yy{nm}", name=f"yy{nm}")
                    nc.vector.tensor_mul(yy, y0, y0)
                    ryy = p2.tile([128, width], F32, tag=f"ry{nm}", name=f"ry{nm}")
                    nc.vector.tensor_mul(ryy, yy, r_in)
                    corr = p2.tile([128, width], F32, tag=f"co{nm}", name=f"co{nm}")
                    nc.vector.tensor_scalar(out=corr, in0=ryy, scalar1=-0.5,
                                            scalar2=1.5,
                                            op0=mybir.AluOpType.mult,
                                            op1=mybir.AluOpType.add)
                    d = p2.tile([128, width], F32, tag=f"d{nm}", name=f"d{nm}")
                    nc.vector.tensor_mul(d, y0, corr)
                    return d

                # own-row d (local r only; runs inside the gather window)
                r8 = p2.tile([n_st, 128], F32)
                nc.scalar.dma_start(
                    r8, r_loc.ap().rearrange("o (s p) -> (o s) p", p=128))
                prT8 = psC.tile([128, n_st], F32, tag="prT8", name="prT8")
                nc.tensor.matmul(prT8, lhsT=r8, rhs=eye32[0:n_st, 0:n_st],
                                 start=True, stop=True)
                d_own = rsqrt_newton(prT8, n_st, "o")

                # full d from the gathered row sums
                rf = p2.tile([num_cores * n_st, 128], F32, tag="rf",
                             name="rf")
                nc.scalar.dma_start(rf, r_ful.ap().rearrange(
                    "c (s p) -> (c s) p", p=128))
                prT = psC.tile([128, n_jb], F32, tag="prT", name="prT")
                nc.tensor.matmul(prT, lhsT=rf, rhs=eye32[0:n_jb, 0:n_jb],
                                 start=True, stop=True)
                d_sb = rsqrt_newton(prT, n_jb, "g")

                def finish_chunk(hc):
                    opre = p2.tile([128, CH2], F32, tag="opre", bufs=2,
                                   name="opre")
                    nc.scalar.copy(opre, pm[hc])
                    for k in range(spg):
                        so = spg * hc + k
                        p2m = psC.tile([128, F], F32, tag="p2m", bufs=2,
                                       name="p2m")
                        nc.tensor.matmul(p2m,
                                         lhsT=opre[:, k * 128:(k + 1) * 128],
                                         rhs=wt_sb, start=True, stop=True)
                        epi = p2.tile([128, F], F32, tag="epi", bufs=2,
                                      name="epi")
                        nc.vector.scalar_tensor_tensor(
                            out=epi, in0=p2m, scalar=d_own[:, so:so + 1],
                            in1=bias_bc, op0=mybir.AluOpType.mult,
                            op1=mybir.AluOpType.add)
                        nc.vector.tensor_scalar_max(out_sb[:, so, :], epi, 0.0)
                        nc.scalar.dma_start(out_ap[:, so, :], out_sb[:, so, :])

                # t-major main matmul (one weight load per stripe)
                for t in range(n_jb):
                    df = p2.tile([128, F], BF16, tag="df", bufs=4, name="df")
                    nc.vector.tensor_scalar(
                        out=df, in0=feat16[:, t, :],
                        scalar1=d_sb[:, t:t + 1],
                        scalar2=None, op0=mybir.AluOpType.mult)
                    for hc in range(2):
                        nc.tensor.matmul(
                            pm[hc], lhsT=df,
                            rhs=adjT[:, t, CH2 * hc:CH2 * (hc + 1)],
                            start=(t == 0), stop=(t == n_jb - 1))
                for hc in range(2):
                    finish_chunk(hc)

    nc.compile()
    return nc


def make_in_maps(adj, features, W, b, P, num_cores):
    """Shard + lay out inputs; adds the +I diagonal into each adjT shard."""
    adj = np.asarray(adj, dtype=np.float32)
    features = np.asarray(features, dtype=np.float32)
    W = np.asarray(W, dtype=np.float32)
    b = np.asarray(b, dtype=np.float32)
    N = adj.shape[0]
    n_jb = N // 128

    feat16 = np.ascontiguousarray(
        features.reshape(n_jb, 128, 128).transpose(1, 0, 2)
    ).astype(ml_dtypes.bfloat16)
    wt = np.ascontiguousarray(W.T)
    bias_b = np.broadcast_to(b[None, :], (128, b.shape[0])).copy()
    ones8 = np.ones((128, 2, 16), dtype=ml_dtypes.float8_e4m3)
    eye32 = np.eye(128, dtype=np.float32)

    in_maps = []
    idx = np.arange(P)
    for c in range(num_cores):
        sh = adj[c * P:(c + 1) * P, :].copy()
        sh[idx, c * P + idx] += 1.0
        at = sh.T.reshape(n_jb, 128, P).transpose(1, 0, 2)
        at8 = np.ascontiguousarray(at).astype(ml_dtypes.float8_e4m3)
        in_maps.append({
            "adjT8": at8,
            "feat16": feat16,
            "wt": wt,
            "bias_b": bias_b,
            "ones8": ones8,
            "eye32": eye32,
        })
    return in_maps


_NC_CACHE = {}


def get_nc(P=N_FULL // NUM_CORES, N=N_FULL, F=F_DIM, num_cores=NUM_CORES):
    key = (P, N, F, num_cores)
    if key not in _NC_CACHE:
        _NC_CACHE[key] = build_kernel(P, N, F, num_cores)
    return _NC_CACHE[key]


def kernel(**inputs):
    adj = np.asarray(inputs["adj"], dtype=np.float32)
    features = np.asarray(inputs["features"], dtype=np.float32)
    W = np.asarray(inputs["W"], dtype=np.float32)
    b = np.asarray(inputs["b"], dtype=np.float32)
    n = adj.shape[0]
    P = n // NUM_CORES
    nc = get_nc(P, n, features.shape[1], NUM_CORES)
    in_maps = make_in_maps(adj, features, W, b, P, NUM_CORES)
    res = run_bass_kernel_spmd(nc, in_maps, core_ids=list(range(NUM_CORES)))
    outs = [np.asarray(res.results[c]["out"], dtype=np.float32)
            for c in range(NUM_CORES)]
    return np.concatenate(outs, axis=0)


# revision 12
# speedup vs baseline: 1.2306x; 1.2306x over previous
"""GCN layer (normalized adjacency @ features -> linear -> relu) on 8 TRN2 NeuronCores.

Strategy (row-sharded, 1D node partition; host does layout/dtype prep only):
  - Host shards adj by rows (P=1024 rows/core), adds the identity diagonal,
    TRANSPOSES the shard to adjT [N, P] and casts to fp8_e4m3 (adj values are
    {0,1,2} -- exact in fp8). Layout [j_within_stripe=128, stripe=64, i=1024]
    so each DMA chunk reads 8KB-contiguous lines per partition. 8MB/core.
  - Features are host-cast to bf16 in [p=128, t=64, f=128] layout (2MB).
  - Device: adjT streams in via HWDGE; the PE computes row sums with a
    DoubleRow fp8 ones-matmul pass (contracts the partition axis, 2 stripes
    per MM) chasing the DMA. Two half-column AllGathers exchange row sums;
    d = rsqrt(r) via Sqrt+reciprocal+Newton. d-scaled bf16 features (lhsT)
    x fp8 adjT (rhs) mixed-dtype matmuls accumulate out_pre.T in PSUM.
  - Epilogue per 512-col chunk: PSUM->SBUF copy, fp32 matmul with W.T
    (host-pretransposed), per-row d scale + bias + relu, per-stripe output DMA.
    Host concatenates the 8 [P,128] outputs.
"""

import numpy as np
import ml_dtypes

import concourse.bass as bass
import concourse.bacc as bacc
import concourse.mybir as mybir
import concourse.tile as tile
from concourse.bass_utils import run_bass_kernel_spmd

F32 = mybir.dt.float32
BF16 = mybir.dt.bfloat16
FP8 = mybir.dt.float8e4

N_FULL = 8192
F_DIM = 128
NUM_CORES = 8


def build_kernel(P=1024, N=8192, F=128, num_cores=8):
    assert P == 1024 and N == 8192 and F == 128
    n_st = P // 128          # 8 output stripes per core
    n_jb = N // 128          # 64 j-stripes (contraction)
    NCH = 4                  # adjT DMA chunks
    jpc = n_jb // NCH        # 8 j-stripes per chunk
    CH2 = 512                # output column chunk (PSUM bank)
    n_g = 2                  # gather groups = local-column halves
    spg = n_st // n_g        # 4 output stripes per group

    nc = bacc.Bacc("TRN2", target_bir_lowering=False, debug=False,
                   num_devices=num_cores)

    adjT_h = nc.declare_dram_parameter("adjT8", [128, n_jb, P], FP8,
                                       isOutput=False)
    feat_h = nc.declare_dram_parameter("feat16", [128, n_jb, F], BF16,
                                       isOutput=False)
    wt_h = nc.declare_dram_parameter("wt", [F, F], BF16, isOutput=False)
    bias_h = nc.declare_dram_parameter("bias_b", [128, F], F32, isOutput=False)
    ones_h = nc.declare_dram_parameter("ones8", [128, 2, 16], FP8,
                                       isOutput=False)
    eye_h = nc.declare_dram_parameter("eye32", [128, 128], F32, isOutput=False)
    out_h = nc.declare_dram_parameter("out", [P, F], F32, isOutput=True)

    r_loc = nc.dram_tensor("r_local", [1, P], F32)
    r_ful = nc.dram_tensor("r_full", [num_cores, P], F32,
                           addr_space="Shared")

    out_ap = out_h.ap().rearrange("(s p) f -> p s f", p=128)

    def gather():
        nc.gpsimd.collective_compute(
            "AllGather", mybir.AluOpType.bypass,
            replica_groups=[list(range(num_cores))],
            ins=[r_loc[:].opt()],
            outs=[r_ful[:].opt()],
        )

    with tile.TileContext(nc) as tc:
        with tc.tile_pool(name="const", bufs=1) as cpool, \
             tc.tile_pool(name="big", bufs=1) as bigp, \
             tc.tile_pool(name="psB", bufs=1, space="PSUM") as psB:

            # ones8 FIRST on the SP ring: same-ring FIFO guarantees it lands
            # before the adjT flood (a parallel ring's small DMAs were
            # observed to starve until the flood drained). Other consts are
            # not needed until after the flood -> scalar ring.
            ones8 = cpool.tile([128, 2, 16], FP8)
            nc.sync.dma_start(ones8, ones_h[:])
            wt_sb = cpool.tile([F, F], BF16)
            nc.scalar.dma_start(wt_sb, wt_h[:])
            bias_bc = cpool.tile([128, F], F32)
            nc.scalar.dma_start(bias_bc, bias_h[:])
            eye32 = cpool.tile([128, 128], F32)
            nc.scalar.dma_start(eye32, eye_h[:])
            # pre-warm the Sqrt activation table (first use pays ~2.7us)
            warm = cpool.tile([1, 1], F32)
            nc.scalar.activation(warm, eye32[0:1, 0:1],
                                 mybir.ActivationFunctionType.Sqrt)

            adjT = bigp.tile([128, n_jb, P], FP8)
            feat16 = bigp.tile([128, n_jb, F], BF16)
            out_sb = bigp.tile([128, n_st, F], F32)

            # main-matmul accumulators (persist across phases)
            pm = [psB.tile([128, CH2], F32, tag=f"pm{h}", name=f"pm{h}")
                  for h in range(2)]

            with tc.tile_pool(name="ph1", bufs=1) as p1, \
                 tc.tile_pool(name="psA", bufs=1, space="PSUM") as psA:
                # adjT chunk DMAs (SP ring) interleaved in program order with
                # the DoubleRow fp8 ones-pass (r[i] = sum_j adjT[j, i], two
                # stripes per MM) so the pass chases the DMA chunk by chunk.
                pr = [psA.tile([1, CH2], F32, tag=f"pr{g}", name=f"pr{g}")
                      for g in range(n_g)]
                # adjT chunk DMAs interleaved with the ones-pass so the
                # pass chases the flood; both column-half accumulations
                # interleave per pair.
                for k in range(NCH):
                    nc.sync.dma_start(adjT[:, k * jpc:(k + 1) * jpc, :],
                                      adjT_h[:, k * jpc:(k + 1) * jpc, :])
                    for pair in range(k * jpc // 2, (k + 1) * jpc // 2):
                        for g in range(n_g):
                            nc.tensor.matmul(
                                pr[g], lhsT=ones8[:, :, 0:1],
                                rhs=adjT[:, 2 * pair:2 * pair + 2,
                                         CH2 * g:CH2 * (g + 1)],
                                start=(pair == 0),
                                stop=(pair == n_jb // 2 - 1),
                                perf_mode=mybir.MatmulPerfMode.DoubleRow)
                # features stream after adjT on the same FIFO ring (needed
                # only once d arrives, well after the gathers fire)
                nc.sync.dma_start(feat16, feat_h[:])
                r_sb = p1.tile([1, P], F32, tag="rsb", name="rsb")
                for g in range(n_g):
                    nc.scalar.copy(r_sb[:, CH2 * g:CH2 * (g + 1)], pr[g])
                nc.scalar.dma_start(r_loc[:], r_sb)
                gather()

            with tc.tile_pool(name="ph2", bufs=1) as p2, \
                 tc.tile_pool(name="psC", bufs=1, space="PSUM") as psC:

                def rsqrt_newton(r_in, width, nm):
                    sq = p2.tile([128, width], F32, tag=f"sq{nm}", name=f"sq{nm}")
                    nc.scalar.activation(sq, r_in,
                                         mybir.ActivationFunctionType.Sqrt)
                    d = p2.tile([128, width], F32, tag=f"d{nm}", name=f"d{nm}")
                    nc.vector.reciprocal(d, sq)
                    return d

                # own-row d (local r only; runs inside the gather window)
                r8 = p2.tile([n_st, 128], F32)
                nc.scalar.dma_start(
                    r8, r_loc.ap().rearrange("o (s p) -> (o s) p", p=128))
                prT8 = psC.tile([128, n_st], F32, tag="prT8", name="prT8")
                nc.tensor.matmul(prT8, lhsT=r8, rhs=eye32[0:n_st, 0:n_st],
                                 start=True, stop=True)
                d_own = rsqrt_newton(prT8, n_st, "o")

                # full d from the gathered row sums
                rf = p2.tile([num_cores * n_st, 128], F32, tag="rf",
                             name="rf")
                nc.scalar.dma_start(rf, r_ful.ap().rearrange(
                    "c (s p) -> (c s) p", p=128))
                prT = psC.tile([128, n_jb], F32, tag="prT", name="prT")
                nc.tensor.matmul(prT, lhsT=rf, rhs=eye32[0:n_jb, 0:n_jb],
                                 start=True, stop=True)
                d_sb = rsqrt_newton(prT, n_jb, "g")

                def finish_chunk(hc):
                    opre = p2.tile([128, CH2], BF16, tag="opre", bufs=2,
                                   name="opre")
                    nc.scalar.copy(opre, pm[hc])
                    for k in range(spg):
                        so = spg * hc + k
                        p2m = psC.tile([128, F], F32, tag="p2m", bufs=2,
                                       name="p2m")
                        nc.tensor.matmul(p2m,
                                         lhsT=opre[:, k * 128:(k + 1) * 128],
                                         rhs=wt_sb, start=True, stop=True)
                        epi = p2.tile([128, F], F32, tag="epi", bufs=2,
                                      name="epi")
                        nc.vector.scalar_tensor_tensor(
                            out=epi, in0=p2m, scalar=d_own[:, so:so + 1],
                            in1=bias_bc, op0=mybir.AluOpType.mult,
                            op1=mybir.AluOpType.add)
                        nc.vector.tensor_scalar_max(out_sb[:, so, :], epi, 0.0)
                    nc.scalar.dma_start(out_ap[:, spg * hc:spg * (hc + 1), :],
                                        out_sb[:, spg * hc:spg * (hc + 1), :])

                # t-major main matmul (one weight load per stripe)
                for t in range(n_jb):
                    df = p2.tile([128, F], BF16, tag="df", bufs=4, name="df")
                    nc.vector.tensor_scalar(
                        out=df, in0=feat16[:, t, :],
                        scalar1=d_sb[:, t:t + 1],
                        scalar2=None, op0=mybir.AluOpType.mult)
                    for hc in range(2):
                        nc.tensor.matmul(
                            pm[hc], lhsT=df,
                            rhs=adjT[:, t, CH2 * hc:CH2 * (hc + 1)],
                            start=(t == 0), stop=(t == n_jb - 1))
                for hc in range(2):
                    finish_chunk(hc)

    nc.compile()
    return nc


def make_in_maps(adj, features, W, b, P, num_cores):
    """Shard + lay out inputs; adds the +I diagonal into each adjT shard."""
    adj = np.asarray(adj, dtype=np.float32)
    features = np.asarray(features, dtype=np.float32)
    W = np.asarray(W, dtype=np.float32)
    b = np.asarray(b, dtype=np.float32)
    N = adj.shape[0]
    n_jb = N // 128

    feat16 = np.ascontiguousarray(
        features.reshape(n_jb, 128, 128).transpose(1, 0, 2)
    ).astype(ml_dtypes.bfloat16)
    wt = np.ascontiguousarray(W.T).astype(ml_dtypes.bfloat16)
    bias_b = np.broadcast_to(b[None, :], (128, b.shape[0])).copy()
    ones8 = np.ones((128, 2, 16), dtype=ml_dtypes.float8_e4m3)
    eye32 = np.eye(128, dtype=np.float32)

    in_maps = []
    idx = np.arange(P)
    for c in range(num_cores):
        sh = adj[c * P:(c + 1) * P, :].copy()
        sh[idx, c * P + idx] += 1.0
        at = sh.T.reshape(n_jb, 128, P).transpose(1, 0, 2)
        at8 = np.ascontiguousarray(at).astype(ml_dtypes.float8_e4m3)
        in_maps.append({
            "adjT8": at8,
            "feat16": feat16,
            "wt": wt,
            "bias_b": bias_b,
            "ones8": ones8,
            "eye32": eye32,
        })
    return in_maps


_NC_CACHE = {}


def get_nc(P=N_FULL // NUM_CORES, N=N_FULL, F=F_DIM, num_cores=NUM_CORES):
    key = (P, N, F, num_cores)
    if key not in _NC_CACHE:
        _NC_CACHE[key] = build_kernel(P, N, F, num_cores)
    return _NC_CACHE[key]


def kernel(**inputs):
    adj = np.asarray(inputs["adj"], dtype=np.float32)
    features = np.asarray(inputs["features"], dtype=np.float32)
    W = np.asarray(inputs["W"], dtype=np.float32)
    b = np.asarray(inputs["b"], dtype=np.float32)
    n = adj.shape[0]
    P = n // NUM_CORES
    nc = get_nc(P, n, features.shape[1], NUM_CORES)
    in_maps = make_in_maps(adj, features, W, b, P, NUM_CORES)
    res = run_bass_kernel_spmd(nc, in_maps, core_ids=list(range(NUM_CORES)))
    outs = [np.asarray(res.results[c]["out"], dtype=np.float32)
            for c in range(NUM_CORES)]
    return np.concatenate(outs, axis=0)
